# revision 6
# baseline (speedup 1.0000x reference)
"""MoE layer (top-2 routing, 8 experts) on 8 Trainium2 NeuronCores.

Sharding: expert-parallel (per the sharding hint). The router is computed on
the host in fp32 (identical math to the reference; measured top-2 logit
margins ~5.7e-5 far exceed fp32 matmul rounding, so the selection matches
exactly). Tokens are then all-to-all'd by top-2 expert assignment: core c
receives the tokens routed to expert c (padded to a fixed capacity C), holds
only expert c's weights, and computes y = W2^T gelu(W1^T x + b1) + b2 for its
token set. The host scatter-adds each expert's output back into the full
[T, DIM] result weighted by the softmaxed router probabilities.

This does 4x less matmul work per core than a dense all-experts approach
(each token visits only K=2 of E=8 experts). FFN matmuls run in bf16 with
fp32 PSUM accumulation; biases + GELU are fused into the PSUM->SBUF copy on
the scalar engine.

Device-side layout per core (capacity C tokens = max expert load, ~2182):
  xin  [ND,128,C]  bf16  x^T tiles (partition dim = d within chunk)
  w1d  [NH,128,ND,128] bf16  W1^T tiles: [hc][128d, dc, 128h] (lhsT)
  w2d  [ND,128,NH,128] bf16  W2^T tiles: [dc][128h, hc, 128d] (lhsT)
  out  [ND,128,C]  f32   y^T
The resident hidden tensor hT [128, NH, C/n_passes] bf16 plus x^T must fit in
SBUF (~208KB/partition); for C <= ~2250 a single pass works (weights are
streamed exactly once), otherwise tokens are processed in two passes.
"""

import sys, os

for _p in ("/root/.axon_site", "/root/.axon_site/_ro/trn_rl_repo",
           "/root/.axon_site/_ro/pypackages", "/opt/trn_rl_repo"):
    if os.path.isdir(_p) and _p not in sys.path:
        sys.path.append(_p)

import numpy as np
import ml_dtypes

BF16 = ml_dtypes.bfloat16

T, DIM, E, K, H = 8192, 1024, 8, 2, 4096
N_CORES = 8
ND = DIM // 128             # 8 d-chunks
NH = H // 128               # 32 h-chunks
C_SINGLE_PASS = 2250        # max capacity for which hT + x^T fit SBUF at once

_compiled = {}


def _groups(n):
    """Split [0, n) into balanced chunks of <=512 (PSUM moving-dim limit)."""
    k = -(-n // 512)
    base, rem = divmod(n, k)
    sizes = [base + (1 if i < rem else 0) for i in range(k)]
    out, o = [], 0
    for s in sizes:
        out.append((o, o + s))
        o += s
    return out


def _build(C):
    from concourse import bass, bacc, tile, mybir

    dt = mybir.dt
    n_passes = 1 if C <= C_SINGLE_PASS else 2
    HALF = C // n_passes
    nc = bacc.Bacc("TRN2", target_bir_lowering=False, debug=False,
                   num_devices=N_CORES)

    xin = nc.dram_tensor("xin", [ND, 128, C], dt.bfloat16, kind="ExternalInput").ap()
    w1d = nc.dram_tensor("w1d", [NH, 128, ND, 128], dt.bfloat16, kind="ExternalInput").ap()
    w2d = nc.dram_tensor("w2d", [ND, 128, NH, 128], dt.bfloat16, kind="ExternalInput").ap()
    b1d = nc.dram_tensor("b1d", [128, NH], dt.float32, kind="ExternalInput").ap()
    b2d = nc.dram_tensor("b2d", [128, ND], dt.float32, kind="ExternalInput").ap()
    out = nc.dram_tensor("out_shard", [ND, 128, C], dt.float32, kind="ExternalOutput").ap()

    with tile.TileContext(nc) as tc:
        with tc.tile_pool(name="const", bufs=1) as const, \
             tc.tile_pool(name="resident", bufs=1) as res, \
             tc.tile_pool(name="w1p", bufs=3) as w1p, \
             tc.tile_pool(name="w2p", bufs=2) as w2p, \
             tc.tile_pool(name="vec", bufs=3) as vec, \
             tc.tile_pool(name="pmm", bufs=4, space="PSUM") as pmm:

            xall = res.tile([128, ND, C], dt.bfloat16)     # x^T, resident
            hT = res.tile([128, NH, HALF], dt.bfloat16)    # hidden for one half
            b1sb = const.tile([128, NH], dt.float32)
            b2sb = const.tile([128, ND], dt.float32)

            nc.sync.dma_start(b1sb[:], b1d[:])
            nc.sync.dma_start(b2sb[:], b2d[:])
            for dc in range(ND):
                nc.sync.dma_start(xall[:, dc, :], xin[dc])

            for half in range(n_passes):
                off = half * HALF
                # ---- layer 1: hT = gelu(x @ W1 + b1), h-major ----
                for hc in range(NH):
                    w1t = w1p.tile([128, ND, 128], dt.bfloat16, tag="w1t")
                    nc.sync.dma_start(w1t[:], w1d[hc])
                    for (g0, g1) in _groups(HALF):
                        ps = pmm.tile([128, g1 - g0], dt.float32,
                                      name=f"ps1_{half}_{hc}_{g0}", tag="ps")
                        for dc in range(ND):
                            nc.tensor.matmul(ps[:], lhsT=w1t[:, dc, :],
                                             rhs=xall[:, dc, off + g0:off + g1],
                                             start=(dc == 0), stop=(dc == ND - 1))
                        nc.scalar.activation(hT[:, hc, g0:g1], ps[:],
                                             bass.mybir.ActivationFunctionType.Gelu,
                                             bias=b1sb[:, hc:hc + 1])
                # ---- layer 2: y = h @ W2 + b2, d-major, straight to DRAM ----
                for dc in range(ND):
                    w2t = w2p.tile([128, NH, 128], dt.bfloat16, tag="w2t")
                    nc.sync.dma_start(w2t[:], w2d[dc])
                    for (g0, g1) in _groups(HALF):
                        ps = pmm.tile([128, g1 - g0], dt.float32,
                                      name=f"ps2_{half}_{dc}_{g0}", tag="ps")
                        for hc in range(NH):
                            nc.tensor.matmul(ps[:], lhsT=w2t[:, hc, :],
                                             rhs=hT[:, hc, g0:g1],
                                             start=(hc == 0), stop=(hc == NH - 1))
                        yo = vec.tile([128, g1 - g0], dt.float32, tag="yo")
                        nc.scalar.activation(yo[:], ps[:],
                                             bass.mybir.ActivationFunctionType.Identity,
                                             bias=b2sb[:, dc:dc + 1])
                        nc.sync.dma_start(out[dc, :, off + g0:off + g1], yo[:])

    nc.compile()
    return nc


def _route(x_flat, Wr):
    """fp32 top-2 routing identical to the reference (argmax twice + softmax)."""
    logits = x_flat @ Wr                                  # [T, E] fp32
    rows = np.arange(T)
    a1 = np.argmax(logits, axis=1)
    l1 = logits[rows, a1]
    tmp = logits.copy()
    tmp[rows, a1] = -np.inf
    a2 = np.argmax(tmp, axis=1)
    l2 = tmp[rows, a2]
    # softmax over the (descending) top-2 values
    p1 = 1.0 / (1.0 + np.exp((l2 - l1).astype(np.float32)))
    p1 = p1.astype(np.float32)
    p2 = (1.0 - p1).astype(np.float32)
    return a1, a2, p1, p2


def kernel(x, Wr, W1, b1, W2, b2, _profile=None):
    global _compiled
    from concourse.bass_utils import run_bass_kernel_spmd

    x_flat = np.ascontiguousarray(np.asarray(x, np.float32)).reshape(T, DIM)
    Wr = np.ascontiguousarray(np.asarray(Wr, np.float32))
    W1 = np.asarray(W1, np.float32)
    b1 = np.asarray(b1, np.float32)
    W2 = np.asarray(W2, np.float32)
    b2 = np.asarray(b2, np.float32)

    a1, a2, p1, p2 = _route(x_flat, Wr)

    # token ids + combine weights per expert
    ids, wts = [], []
    for e in range(E):
        m1 = np.nonzero(a1 == e)[0]
        m2 = np.nonzero(a2 == e)[0]
        ids.append(np.concatenate([m1, m2]))
        wts.append(np.concatenate([p1[m1], p2[m2]]).astype(np.float32))

    max_n = max(len(i) for i in ids)
    C = max(512, max_n + (max_n % 2))            # capacity = max expert load, even
    if C not in _compiled:
        _compiled[C] = _build(C)
    nc = _compiled[C]

    # per-expert weight tiles (lhsT layouts; see module docstring)
    w1d = np.ascontiguousarray(
        W1.astype(BF16).reshape(E, ND, 128, NH, 128).transpose(0, 3, 2, 1, 4))
    w2d = np.ascontiguousarray(
        W2.astype(BF16).reshape(E, NH, 128, ND, 128).transpose(0, 3, 2, 1, 4))
    b1d = np.ascontiguousarray(b1.reshape(E, NH, 128).transpose(0, 2, 1))
    b2d = np.ascontiguousarray(b2.reshape(E, ND, 128).transpose(0, 2, 1))

    in_maps = []
    for e in range(E):
        xg = np.zeros((C, DIM), np.float32)
        xg[:len(ids[e])] = x_flat[ids[e]]
        xT = np.ascontiguousarray(xg.T).astype(BF16).reshape(ND, 128, C)
        in_maps.append({
            "xin": xT,
            "w1d": w1d[e],
            "w2d": w2d[e],
            "b1d": b1d[e],
            "b2d": b2d[e],
        })

    kwargs = {}
    if _profile:
        kwargs = dict(trace=True, tmpdir=_profile)
    res = run_bass_kernel_spmd(nc, in_maps, core_ids=list(range(N_CORES)), **kwargs)

    out_full = np.zeros((T, DIM), np.float32)
    for e in range(E):
        n = len(ids[e])
        yT = np.asarray(res.results[e]["out_shard"], np.float32).reshape(DIM, C)
        out_full[ids[e]] += wts[e][:, None] * yT[:, :n].T

    full = out_full.reshape(4, 2048, DIM)
    if _profile:
        return full, res
    return full


# revision 7
# speedup vs baseline: 1.1467x; 1.1467x over previous
"""MoE layer (top-2 routing, 8 experts) on 8 Trainium2 NeuronCores.

Sharding: expert-parallel (per the sharding hint). The router is computed on
the host in fp32 (identical math to the reference; measured top-2 logit
margins ~5.7e-5 far exceed fp32 matmul rounding, so the selection matches
exactly). Tokens are then all-to-all'd by top-2 expert assignment: core c
receives the tokens routed to expert c (padded to a fixed capacity C), holds
only expert c's weights, and computes y = W2^T gelu(W1^T x + b1) + b2 for its
token set. The host scatter-adds each expert's output back into the full
[T, DIM] result weighted by the softmaxed router probabilities.

This does 4x less matmul work per core than a dense all-experts approach
(each token visits only K=2 of E=8 experts). FFN matmuls run in bf16 with
fp32 PSUM accumulation; biases + GELU are fused into the PSUM->SBUF copy on
the scalar engine.

Device-side layout per core (capacity C tokens = max expert load, ~2182):
  xin  [ND,128,C]  bf16  x^T tiles (partition dim = d within chunk)
  w1d  [NH,128,ND,128] bf16  W1^T tiles: [hc][128d, dc, 128h] (lhsT)
  w2d  [ND,128,NH,128] bf16  W2^T tiles: [dc][128h, hc, 128d] (lhsT)
  out  [ND,128,C]  f32   y^T
The resident hidden tensor hT [128, NH, C/n_passes] bf16 plus x^T must fit in
SBUF (~208KB/partition); for C <= ~2250 a single pass works (weights are
streamed exactly once), otherwise tokens are processed in two passes.
"""

import sys, os

for _p in ("/root/.axon_site", "/root/.axon_site/_ro/trn_rl_repo",
           "/root/.axon_site/_ro/pypackages", "/opt/trn_rl_repo"):
    if os.path.isdir(_p) and _p not in sys.path:
        sys.path.append(_p)

import numpy as np
import ml_dtypes

BF16 = ml_dtypes.bfloat16

T, DIM, E, K, H = 8192, 1024, 8, 2, 4096
N_CORES = 8
ND = DIM // 128             # 8 d-chunks
NH = H // 128               # 32 h-chunks
C_SINGLE_PASS = 2250        # max capacity for which hT + x^T fit SBUF at once

_compiled = {}


def _groups(n):
    """Split [0, n) into balanced chunks of <=512 (PSUM moving-dim limit)."""
    k = -(-n // 512)
    base, rem = divmod(n, k)
    sizes = [base + (1 if i < rem else 0) for i in range(k)]
    out, o = [], 0
    for s in sizes:
        out.append((o, o + s))
        o += s
    return out


def _build(C):
    from concourse import bass, bacc, tile, mybir

    dt = mybir.dt
    n_passes = 1 if C <= C_SINGLE_PASS else 2
    HALF = C // n_passes
    nc = bacc.Bacc("TRN2", target_bir_lowering=False, debug=False,
                   num_devices=N_CORES)

    xin = nc.dram_tensor("xin", [ND, 128, C], dt.bfloat16, kind="ExternalInput").ap()
    w1d = nc.dram_tensor("w1d", [NH, 128, ND, 128], dt.bfloat16, kind="ExternalInput").ap()
    w2d = nc.dram_tensor("w2d", [ND, 128, NH, 128], dt.bfloat16, kind="ExternalInput").ap()
    b1d = nc.dram_tensor("b1d", [128, NH], dt.float32, kind="ExternalInput").ap()
    b2d = nc.dram_tensor("b2d", [128, ND], dt.float32, kind="ExternalInput").ap()
    out = nc.dram_tensor("out_shard", [ND, 128, C], dt.float32, kind="ExternalOutput").ap()

    with tile.TileContext(nc) as tc:
        with tc.tile_pool(name="const", bufs=1) as const, \
             tc.tile_pool(name="resident", bufs=1) as res, \
             tc.tile_pool(name="w1p", bufs=3) as w1p, \
             tc.tile_pool(name="w2p", bufs=2) as w2p, \
             tc.tile_pool(name="vec", bufs=3) as vec, \
             tc.tile_pool(name="pmm", bufs=8, space="PSUM") as pmm:

            xall = res.tile([128, ND, C], dt.bfloat16)     # x^T, resident
            hT = res.tile([128, NH, HALF], dt.bfloat16)    # hidden for one half
            b1sb = const.tile([128, NH], dt.float32)
            b2sb = const.tile([128, ND], dt.float32)

            nc.sync.dma_start(b1sb[:], b1d[:])
            nc.sync.dma_start(b2sb[:], b2d[:])
            # group-major x loads so the first L1 psum group's operands land
            # quickly (matmuls only wait on the sub-regions they read)
            for (g0, g1) in _groups(C):
                for dc in range(ND):
                    nc.sync.dma_start(xall[:, dc, g0:g1], xin[dc, :, g0:g1])

            for half in range(n_passes):
                off = half * HALF
                grps = _groups(HALF)
                # ---- layer 1: hT = gelu(x @ W1 + b1), h-major ----
                # dc is the OUTER loop over token groups: the stationary weight
                # tile w1t[:, dc, :] is reused across all groups, so the PE
                # only switches weights 8x per hc instead of every matmul.
                for hc in range(NH):
                    w1t = w1p.tile([128, ND, 128], dt.bfloat16, tag="w1t")
                    nc.sync.dma_start(w1t[:], w1d[hc])
                    pss = [pmm.tile([128, g1 - g0], dt.float32,
                                    name=f"ps1_{half}_{hc}_{g0}", tag="ps")
                           for (g0, g1) in grps]
                    for dc in range(ND):
                        for gi, (g0, g1) in enumerate(grps):
                            nc.tensor.matmul(pss[gi][:], lhsT=w1t[:, dc, :],
                                             rhs=xall[:, dc, off + g0:off + g1],
                                             start=(dc == 0), stop=(dc == ND - 1))
                    for gi, (g0, g1) in enumerate(grps):
                        nc.scalar.activation(hT[:, hc, g0:g1], pss[gi][:],
                                             bass.mybir.ActivationFunctionType.Gelu,
                                             bias=b1sb[:, hc:hc + 1])
                # ---- layer 2: y = h @ W2 + b2, d-major, straight to DRAM ----
                for dc in range(ND):
                    w2t = w2p.tile([128, NH, 128], dt.bfloat16, tag="w2t")
                    nc.sync.dma_start(w2t[:], w2d[dc])
                    pss = [pmm.tile([128, g1 - g0], dt.float32,
                                    name=f"ps2_{half}_{dc}_{g0}", tag="ps")
                           for (g0, g1) in grps]
                    for hc in range(NH):
                        for gi, (g0, g1) in enumerate(grps):
                            nc.tensor.matmul(pss[gi][:], lhsT=w2t[:, hc, :],
                                             rhs=hT[:, hc, g0:g1],
                                             start=(hc == 0), stop=(hc == NH - 1))
                    for gi, (g0, g1) in enumerate(grps):
                        yo = vec.tile([128, g1 - g0], dt.float32, tag="yo")
                        nc.scalar.activation(yo[:], pss[gi][:],
                                             bass.mybir.ActivationFunctionType.Identity,
                                             bias=b2sb[:, dc:dc + 1])
                        nc.sync.dma_start(out[dc, :, off + g0:off + g1], yo[:])

    nc.compile()
    return nc


def _route(x_flat, Wr):
    """fp32 top-2 routing identical to the reference (argmax twice + softmax)."""
    logits = x_flat @ Wr                                  # [T, E] fp32
    rows = np.arange(T)
    a1 = np.argmax(logits, axis=1)
    l1 = logits[rows, a1]
    tmp = logits.copy()
    tmp[rows, a1] = -np.inf
    a2 = np.argmax(tmp, axis=1)
    l2 = tmp[rows, a2]
    # softmax over the (descending) top-2 values
    p1 = 1.0 / (1.0 + np.exp((l2 - l1).astype(np.float32)))
    p1 = p1.astype(np.float32)
    p2 = (1.0 - p1).astype(np.float32)
    return a1, a2, p1, p2


def kernel(x, Wr, W1, b1, W2, b2, _profile=None):
    global _compiled
    from concourse.bass_utils import run_bass_kernel_spmd

    x_flat = np.ascontiguousarray(np.asarray(x, np.float32)).reshape(T, DIM)
    Wr = np.ascontiguousarray(np.asarray(Wr, np.float32))
    W1 = np.asarray(W1, np.float32)
    b1 = np.asarray(b1, np.float32)
    W2 = np.asarray(W2, np.float32)
    b2 = np.asarray(b2, np.float32)

    a1, a2, p1, p2 = _route(x_flat, Wr)

    # token ids + combine weights per expert
    ids, wts = [], []
    for e in range(E):
        m1 = np.nonzero(a1 == e)[0]
        m2 = np.nonzero(a2 == e)[0]
        ids.append(np.concatenate([m1, m2]))
        wts.append(np.concatenate([p1[m1], p2[m2]]).astype(np.float32))

    max_n = max(len(i) for i in ids)
    C = max(512, max_n + (max_n % 2))            # capacity = max expert load, even
    if C not in _compiled:
        _compiled[C] = _build(C)
    nc = _compiled[C]

    # per-expert weight tiles (lhsT layouts; see module docstring)
    w1d = np.ascontiguousarray(
        W1.astype(BF16).reshape(E, ND, 128, NH, 128).transpose(0, 3, 2, 1, 4))
    w2d = np.ascontiguousarray(
        W2.astype(BF16).reshape(E, NH, 128, ND, 128).transpose(0, 3, 2, 1, 4))
    b1d = np.ascontiguousarray(b1.reshape(E, NH, 128).transpose(0, 2, 1))
    b2d = np.ascontiguousarray(b2.reshape(E, ND, 128).transpose(0, 2, 1))

    in_maps = []
    for e in range(E):
        xg = np.zeros((C, DIM), np.float32)
        xg[:len(ids[e])] = x_flat[ids[e]]
        xT = np.ascontiguousarray(xg.T).astype(BF16).reshape(ND, 128, C)
        in_maps.append({
            "xin": xT,
            "w1d": w1d[e],
            "w2d": w2d[e],
            "b1d": b1d[e],
            "b2d": b2d[e],
        })

    kwargs = {}
    if _profile:
        kwargs = dict(trace=True, tmpdir=_profile)
    res = run_bass_kernel_spmd(nc, in_maps, core_ids=list(range(N_CORES)), **kwargs)

    out_full = np.zeros((T, DIM), np.float32)
    for e in range(E):
        n = len(ids[e])
        yT = np.asarray(res.results[e]["out_shard"], np.float32).reshape(DIM, C)
        out_full[ids[e]] += wts[e][:, None] * yT[:, :n].T

    full = out_full.reshape(4, 2048, DIM)
    if _profile:
        return full, res
    return full


# revision 8
# speedup vs baseline: 1.1941x; 1.0413x over previous
"""MoE layer (top-2 routing, 8 experts) on 8 Trainium2 NeuronCores.

Sharding: expert-parallel (per the sharding hint). The router is computed on
the host in fp32 (identical math to the reference; measured top-2 logit
margins ~5.7e-5 far exceed fp32 matmul rounding, so the selection matches
exactly). Tokens are then all-to-all'd by top-2 expert assignment: core c
receives the tokens routed to expert c (padded to a fixed capacity C), holds
only expert c's weights, and computes y = W2^T gelu(W1^T x + b1) + b2 for its
token set. The host scatter-adds each expert's output back into the full
[T, DIM] result weighted by the softmaxed router probabilities.

This does 4x less matmul work per core than a dense all-experts approach
(each token visits only K=2 of E=8 experts). FFN matmuls run in bf16 with
fp32 PSUM accumulation; biases + GELU are fused into the PSUM->SBUF copy on
the scalar engine.

Device-side layout per core (capacity C tokens = max expert load, ~2182):
  xin  [ND,128,C]  bf16  x^T tiles (partition dim = d within chunk)
  w1d  [NH,128,ND,128] bf16  W1^T tiles: [hc][128d, dc, 128h] (lhsT)
  w2d  [ND,128,NH,128] bf16  W2^T tiles: [dc][128h, hc, 128d] (lhsT)
  out  [ND,128,C]  f32   y^T
The resident hidden tensor hT [128, NH, C/n_passes] bf16 plus x^T must fit in
SBUF (~208KB/partition); for C <= ~2250 a single pass works (weights are
streamed exactly once), otherwise tokens are processed in two passes.
"""

import sys, os

for _p in ("/root/.axon_site", "/root/.axon_site/_ro/trn_rl_repo",
           "/root/.axon_site/_ro/pypackages", "/opt/trn_rl_repo"):
    if os.path.isdir(_p) and _p not in sys.path:
        sys.path.append(_p)

import numpy as np
import ml_dtypes

BF16 = ml_dtypes.bfloat16

T, DIM, E, K, H = 8192, 1024, 8, 2, 4096
N_CORES = 8
ND = DIM // 128             # 8 d-chunks
NH = H // 128               # 32 h-chunks
C_SINGLE_PASS = 2250        # max capacity for which hT + x^T fit SBUF at once

_compiled = {}


def _groups(n):
    """Split [0, n) into balanced chunks of <=512 (PSUM moving-dim limit)."""
    k = -(-n // 512)
    base, rem = divmod(n, k)
    sizes = [base + (1 if i < rem else 0) for i in range(k)]
    out, o = [], 0
    for s in sizes:
        out.append((o, o + s))
        o += s
    return out


def _build(C):
    from concourse import bass, bacc, tile, mybir

    dt = mybir.dt
    n_passes = 1 if C <= C_SINGLE_PASS else 2
    HALF = C // n_passes
    nc = bacc.Bacc("TRN2", target_bir_lowering=False, debug=False,
                   num_devices=N_CORES)

    xin = nc.dram_tensor("xin", [ND, 128, C], dt.bfloat16, kind="ExternalInput").ap()
    w1d = nc.dram_tensor("w1d", [NH, 128, ND, 128], dt.bfloat16, kind="ExternalInput").ap()
    w2d = nc.dram_tensor("w2d", [ND, 128, NH, 128], dt.bfloat16, kind="ExternalInput").ap()
    b1d = nc.dram_tensor("b1d", [128, NH], dt.float32, kind="ExternalInput").ap()
    b2d = nc.dram_tensor("b2d", [128, ND], dt.float32, kind="ExternalInput").ap()
    out = nc.dram_tensor("out_shard", [ND, 128, C], dt.float32, kind="ExternalOutput").ap()

    with tile.TileContext(nc) as tc:
        with tc.tile_pool(name="const", bufs=1) as const, \
             tc.tile_pool(name="resident", bufs=1) as res, \
             tc.tile_pool(name="w1p", bufs=3) as w1p, \
             tc.tile_pool(name="w2p", bufs=2) as w2p, \
             tc.tile_pool(name="vec", bufs=3) as vec, \
             tc.tile_pool(name="pmm", bufs=8, space="PSUM") as pmm:

            xall = res.tile([128, ND, C], dt.bfloat16)     # x^T, resident
            hT = res.tile([128, NH, HALF], dt.bfloat16)    # hidden for one half
            b1sb = const.tile([128, NH], dt.float32)
            b2sb = const.tile([128, ND], dt.float32)

            nc.sync.dma_start(b1sb[:], b1d[:])
            nc.sync.dma_start(b2sb[:], b2d[:])
            # first weight tile queued ahead of the bulk x load so the PE can
            # start as soon as x lands (x DMAs use full-row 2C-byte lines for
            # max HBM bandwidth)
            w1t0 = w1p.tile([128, ND, 128], dt.bfloat16, tag="w1t")
            nc.sync.dma_start(w1t0[:], w1d[0])
            for dc in range(ND):
                nc.sync.dma_start(xall[:, dc, :], xin[dc])

            for half in range(n_passes):
                off = half * HALF
                grps = _groups(HALF)
                # ---- layer 1: hT = gelu(x @ W1 + b1), h-major ----
                # dc is the OUTER loop over token groups: the stationary weight
                # tile w1t[:, dc, :] is reused across all groups, so the PE
                # only switches weights 8x per hc instead of every matmul.
                for hc in range(NH):
                    if half == 0 and hc == 0:
                        w1t = w1t0
                    else:
                        w1t = w1p.tile([128, ND, 128], dt.bfloat16, tag="w1t")
                        nc.sync.dma_start(w1t[:], w1d[hc])
                    pss = [pmm.tile([128, g1 - g0], dt.float32,
                                    name=f"ps1_{half}_{hc}_{g0}", tag="ps")
                           for (g0, g1) in grps]
                    for dc in range(ND):
                        for gi, (g0, g1) in enumerate(grps):
                            nc.tensor.matmul(pss[gi][:], lhsT=w1t[:, dc, :],
                                             rhs=xall[:, dc, off + g0:off + g1],
                                             start=(dc == 0), stop=(dc == ND - 1))
                    for gi, (g0, g1) in enumerate(grps):
                        nc.scalar.activation(hT[:, hc, g0:g1], pss[gi][:],
                                             bass.mybir.ActivationFunctionType.Gelu,
                                             bias=b1sb[:, hc:hc + 1])
                # ---- layer 2: y = h @ W2 + b2, d-major, straight to DRAM ----
                for dc in range(ND):
                    w2t = w2p.tile([128, NH, 128], dt.bfloat16, tag="w2t")
                    nc.sync.dma_start(w2t[:], w2d[dc])
                    pss = [pmm.tile([128, g1 - g0], dt.float32,
                                    name=f"ps2_{half}_{dc}_{g0}", tag="ps")
                           for (g0, g1) in grps]
                    for hc in range(NH):
                        for gi, (g0, g1) in enumerate(grps):
                            nc.tensor.matmul(pss[gi][:], lhsT=w2t[:, hc, :],
                                             rhs=hT[:, hc, g0:g1],
                                             start=(hc == 0), stop=(hc == NH - 1))
                    for gi, (g0, g1) in enumerate(grps):
                        yo = vec.tile([128, g1 - g0], dt.float32, tag="yo")
                        nc.scalar.activation(yo[:], pss[gi][:],
                                             bass.mybir.ActivationFunctionType.Identity,
                                             bias=b2sb[:, dc:dc + 1])
                        nc.sync.dma_start(out[dc, :, off + g0:off + g1], yo[:])

    nc.compile()
    return nc


def _route(x_flat, Wr):
    """fp32 top-2 routing identical to the reference (argmax twice + softmax)."""
    logits = x_flat @ Wr                                  # [T, E] fp32
    rows = np.arange(T)
    a1 = np.argmax(logits, axis=1)
    l1 = logits[rows, a1]
    tmp = logits.copy()
    tmp[rows, a1] = -np.inf
    a2 = np.argmax(tmp, axis=1)
    l2 = tmp[rows, a2]
    # softmax over the (descending) top-2 values
    p1 = 1.0 / (1.0 + np.exp((l2 - l1).astype(np.float32)))
    p1 = p1.astype(np.float32)
    p2 = (1.0 - p1).astype(np.float32)
    return a1, a2, p1, p2


def kernel(x, Wr, W1, b1, W2, b2, _profile=None):
    global _compiled
    from concourse.bass_utils import run_bass_kernel_spmd

    x_flat = np.ascontiguousarray(np.asarray(x, np.float32)).reshape(T, DIM)
    Wr = np.ascontiguousarray(np.asarray(Wr, np.float32))
    W1 = np.asarray(W1, np.float32)
    b1 = np.asarray(b1, np.float32)
    W2 = np.asarray(W2, np.float32)
    b2 = np.asarray(b2, np.float32)

    a1, a2, p1, p2 = _route(x_flat, Wr)

    # token ids + combine weights per expert
    ids, wts = [], []
    for e in range(E):
        m1 = np.nonzero(a1 == e)[0]
        m2 = np.nonzero(a2 == e)[0]
        ids.append(np.concatenate([m1, m2]))
        wts.append(np.concatenate([p1[m1], p2[m2]]).astype(np.float32))

    max_n = max(len(i) for i in ids)
    C = max(512, max_n + (max_n % 2))            # capacity = max expert load, even
    if C not in _compiled:
        _compiled[C] = _build(C)
    nc = _compiled[C]

    # per-expert weight tiles (lhsT layouts; see module docstring)
    w1d = np.ascontiguousarray(
        W1.astype(BF16).reshape(E, ND, 128, NH, 128).transpose(0, 3, 2, 1, 4))
    w2d = np.ascontiguousarray(
        W2.astype(BF16).reshape(E, NH, 128, ND, 128).transpose(0, 3, 2, 1, 4))
    b1d = np.ascontiguousarray(b1.reshape(E, NH, 128).transpose(0, 2, 1))
    b2d = np.ascontiguousarray(b2.reshape(E, ND, 128).transpose(0, 2, 1))

    in_maps = []
    for e in range(E):
        xg = np.zeros((C, DIM), np.float32)
        xg[:len(ids[e])] = x_flat[ids[e]]
        xT = np.ascontiguousarray(xg.T).astype(BF16).reshape(ND, 128, C)
        in_maps.append({
            "xin": xT,
            "w1d": w1d[e],
            "w2d": w2d[e],
            "b1d": b1d[e],
            "b2d": b2d[e],
        })

    kwargs = {}
    if _profile:
        kwargs = dict(trace=True, tmpdir=_profile)
    res = run_bass_kernel_spmd(nc, in_maps, core_ids=list(range(N_CORES)), **kwargs)

    out_full = np.zeros((T, DIM), np.float32)
    for e in range(E):
        n = len(ids[e])
        yT = np.asarray(res.results[e]["out_shard"], np.float32).reshape(DIM, C)
        out_full[ids[e]] += wts[e][:, None] * yT[:, :n].T

    full = out_full.reshape(4, 2048, DIM)
    if _profile:
        return full, res
    return full


# revision 12
# speedup vs baseline: 1.2001x; 1.0051x over previous
"""MoE layer (top-2 routing, 8 experts) on 8 Trainium2 NeuronCores.

Sharding: expert-parallel (per the sharding hint). The router is computed on
the host in fp32 (identical math to the reference; measured top-2 logit
margins ~5.7e-5 far exceed fp32 matmul rounding, so the selection matches
exactly). Tokens are then all-to-all'd by top-2 expert assignment: core c
receives the tokens routed to expert c (padded to a fixed capacity C), holds
only expert c's weights, and computes y = W2^T gelu(W1^T x + b1) + b2 for its
token set. The host scatter-adds each expert's output back into the full
[T, DIM] result weighted by the softmaxed router probabilities.

This does 4x less matmul work per core than a dense all-experts approach
(each token visits only K=2 of E=8 experts). FFN matmuls run in bf16 with
fp32 PSUM accumulation; biases + GELU are fused into the PSUM->SBUF copy on
the scalar engine.

Device-side layout per core (capacity C = NG*G tokens, G<=512, ~2185):
  xin  [NG,128,ND,G] bf16  x^T in token blocks (7KB DMA lines, so the PE can
                           start on block 0 while later blocks stream in)
  w1d  [NH,128,ND,128] bf16  W1^T tiles: [hc][128d, dc, 128h] (lhsT)
  w2d  [ND,128,NH,128] bf16  W2^T tiles: [dc][128h, hc, 128d] (lhsT)
  out  [ND,128,C]  f32   y^T
Inside each layer the stationary weight tile streams all NG token blocks
back-to-back (PSUM banks round-robin across blocks, which also avoids
same-bank accumulation turnaround stalls). The resident hidden tensor
hT [128,NH,C/n_passes] bf16 plus x^T must fit SBUF (~208KB/partition); for
C <= ~2250 a single pass works (weights are streamed exactly once),
otherwise tokens are processed in two passes of half the blocks.
"""

import sys, os

for _p in ("/root/.axon_site", "/root/.axon_site/_ro/trn_rl_repo",
           "/root/.axon_site/_ro/pypackages", "/opt/trn_rl_repo"):
    if os.path.isdir(_p) and _p not in sys.path:
        sys.path.append(_p)

import numpy as np
import ml_dtypes

BF16 = ml_dtypes.bfloat16

T, DIM, E, K, H = 8192, 1024, 8, 2, 4096
N_CORES = 8
ND = DIM // 128             # 8 d-chunks
NH = H // 128               # 32 h-chunks
C_SINGLE_PASS = 2250        # max capacity for which hT + x^T fit SBUF at once

_compiled = {}


def _block_plan(max_n):
    """Capacity C = NG*G with G <= 512; pass ranges over blocks."""
    c_raw = max(512, max_n)
    ng = -(-c_raw // 512)
    g = -(-c_raw // ng)
    C = ng * g
    if C <= C_SINGLE_PASS:
        passes = [(0, ng)]
    else:
        nb1 = -(-ng // 2)
        passes = [(0, nb1), (nb1, ng)]
    return C, ng, g, passes


def _build(C):
    from concourse import bass, bacc, tile, mybir

    dt = mybir.dt
    C, NG, G, passes = _block_plan(C)
    NBmax = max(b1 - b0 for (b0, b1) in passes)
    nc = bacc.Bacc("TRN2", target_bir_lowering=False, debug=False,
                   num_devices=N_CORES)

    xin = nc.dram_tensor("xin", [NG, 128, ND, G], dt.bfloat16, kind="ExternalInput").ap()
    w1d = nc.dram_tensor("w1d", [NH, 128, ND, 128], dt.bfloat16, kind="ExternalInput").ap()
    w2d = nc.dram_tensor("w2d", [ND, 128, NH, 128], dt.bfloat16, kind="ExternalInput").ap()
    b1d = nc.dram_tensor("b1d", [128, NH], dt.float32, kind="ExternalInput").ap()
    b2d = nc.dram_tensor("b2d", [128, ND], dt.float32, kind="ExternalInput").ap()
    out = nc.dram_tensor("out_shard", [ND, 128, C], dt.float32, kind="ExternalOutput").ap()

    with tile.TileContext(nc) as tc:
        with tc.tile_pool(name="const", bufs=1) as const, \
             tc.tile_pool(name="resident", bufs=1) as res, \
             tc.tile_pool(name="w1p", bufs=3) as w1p, \
             tc.tile_pool(name="w2p", bufs=2) as w2p, \
             tc.tile_pool(name="vec", bufs=6) as vec, \
             tc.tile_pool(name="pmm", bufs=8, space="PSUM") as pmm:

            xall = res.tile([128, NG, ND, G], dt.bfloat16)   # x^T, resident
            hT = res.tile([128, NH, NBmax * G], dt.bfloat16)
            b1sb = const.tile([128, NH], dt.float32)
            b2sb = const.tile([128, ND], dt.float32)

            nc.sync.dma_start(b1sb[:], b1d[:])
            nc.sync.dma_start(b2sb[:], b2d[:])
            # first weight tile ahead of the bulk x load; x arrives block by
            # block so the PE can start on block 0 almost immediately
            w1t0 = w1p.tile([128, ND, 128], dt.bfloat16, tag="w1t")
            nc.sync.dma_start(w1t0[:], w1d[0])
            for b in range(NG):
                nc.sync.dma_start(xall[:, b], xin[b])

            for pi, (b0, b1) in enumerate(passes):
                nb = b1 - b0
                # ---- layer 1: hT = gelu(x @ W1 + b1), h-major ----
                # dc is the OUTER loop over token blocks: the stationary
                # weight tile w1t[:, dc, :] streams all blocks back-to-back
                # (weight switches 8x per hc; PSUM banks round-robin).
                for hc in range(NH):
                    if pi == 0 and hc == 0:
                        w1t = w1t0
                    else:
                        w1t = w1p.tile([128, ND, 128], dt.bfloat16, tag="w1t")
                        nc.sync.dma_start(w1t[:], w1d[hc])
                    pss = [pmm.tile([128, G], dt.float32,
                                    name=f"ps1_{pi}_{hc}_{bi}", tag="ps")
                           for bi in range(nb)]
                    for dc in range(ND):
                        for bi in range(nb):
                            nc.tensor.matmul(pss[bi][:], lhsT=w1t[:, dc, :],
                                             rhs=xall[:, b0 + bi, dc, :],
                                             start=(dc == 0), stop=(dc == ND - 1))
                    for bi in range(nb):
                        nc.scalar.activation(hT[:, hc, bi * G:(bi + 1) * G],
                                             pss[bi][:],
                                             bass.mybir.ActivationFunctionType.Gelu,
                                             bias=b1sb[:, hc:hc + 1])
                # ---- layer 2: y = h @ W2 + b2, d-major, straight to DRAM ----
                for dc in range(ND):
                    w2t = w2p.tile([128, NH, 128], dt.bfloat16, tag="w2t")
                    nc.sync.dma_start(w2t[:], w2d[dc])
                    pss = [pmm.tile([128, G], dt.float32,
                                    name=f"ps2_{pi}_{dc}_{bi}", tag="ps")
                           for bi in range(nb)]
                    for hc in range(NH):
                        for bi in range(nb):
                            nc.tensor.matmul(pss[bi][:], lhsT=w2t[:, hc, :],
                                             rhs=hT[:, hc, bi * G:(bi + 1) * G],
                                             start=(hc == 0), stop=(hc == NH - 1))
                    for bi in range(nb):
                        yo = vec.tile([128, G], dt.float32, tag="yo")
                        nc.scalar.activation(yo[:], pss[bi][:],
                                             bass.mybir.ActivationFunctionType.Identity,
                                             bias=b2sb[:, dc:dc + 1])
                        nc.sync.dma_start(out[dc, :, (b0 + bi) * G:(b0 + bi + 1) * G],
                                          yo[:])

    nc.compile()
    return nc


def _route(x_flat, Wr):
    """fp32 top-2 routing identical to the reference (argmax twice + softmax)."""
    logits = x_flat @ Wr                                  # [T, E] fp32
    rows = np.arange(T)
    a1 = np.argmax(logits, axis=1)
    l1 = logits[rows, a1]
    tmp = logits.copy()
    tmp[rows, a1] = -np.inf
    a2 = np.argmax(tmp, axis=1)
    l2 = tmp[rows, a2]
    # softmax over the (descending) top-2 values
    p1 = 1.0 / (1.0 + np.exp((l2 - l1).astype(np.float32)))
    p1 = p1.astype(np.float32)
    p2 = (1.0 - p1).astype(np.float32)
    return a1, a2, p1, p2


def kernel(x, Wr, W1, b1, W2, b2, _profile=None):
    global _compiled
    from concourse.bass_utils import run_bass_kernel_spmd

    x_flat = np.ascontiguousarray(np.asarray(x, np.float32)).reshape(T, DIM)
    Wr = np.ascontiguousarray(np.asarray(Wr, np.float32))
    W1 = np.asarray(W1, np.float32)
    b1 = np.asarray(b1, np.float32)
    W2 = np.asarray(W2, np.float32)
    b2 = np.asarray(b2, np.float32)

    a1, a2, p1, p2 = _route(x_flat, Wr)

    # token ids + combine weights per expert
    ids, wts = [], []
    for e in range(E):
        m1 = np.nonzero(a1 == e)[0]
        m2 = np.nonzero(a2 == e)[0]
        ids.append(np.concatenate([m1, m2]))
        wts.append(np.concatenate([p1[m1], p2[m2]]).astype(np.float32))

    max_n = max(len(i) for i in ids)
    C, NG, G, _passes = _block_plan(max_n)       # capacity >= max expert load
    if C not in _compiled:
        _compiled[C] = _build(C)
    nc = _compiled[C]

    # per-expert weight tiles (lhsT layouts; see module docstring)
    w1d = np.ascontiguousarray(
        W1.astype(BF16).reshape(E, ND, 128, NH, 128).transpose(0, 3, 2, 1, 4))
    w2d = np.ascontiguousarray(
        W2.astype(BF16).reshape(E, NH, 128, ND, 128).transpose(0, 3, 2, 1, 4))
    b1d = np.ascontiguousarray(b1.reshape(E, NH, 128).transpose(0, 2, 1))
    b2d = np.ascontiguousarray(b2.reshape(E, ND, 128).transpose(0, 2, 1))

    in_maps = []
    for e in range(E):
        xg = np.zeros((C, DIM), np.float32)
        xg[:len(ids[e])] = x_flat[ids[e]]
        # [NG, 128part, ND, G] token blocks (7KB per-partition DMA lines)
        xT = np.ascontiguousarray(
            xg.T.reshape(ND, 128, NG, G).transpose(2, 1, 0, 3)).astype(BF16)
        in_maps.append({
            "xin": xT,
            "w1d": w1d[e],
            "w2d": w2d[e],
            "b1d": b1d[e],
            "b2d": b2d[e],
        })

    kwargs = {}
    if _profile:
        kwargs = dict(trace=True, tmpdir=_profile)
    res = run_bass_kernel_spmd(nc, in_maps, core_ids=list(range(N_CORES)), **kwargs)

    out_full = np.zeros((T, DIM), np.float32)
    for e in range(E):
        n = len(ids[e])
        yT = np.asarray(res.results[e]["out_shard"], np.float32).reshape(DIM, C)
        out_full[ids[e]] += wts[e][:, None] * yT[:, :n].T

    full = out_full.reshape(4, 2048, DIM)
    if _profile:
        return full, res
    return full
